# revision 19
# baseline (speedup 1.0000x reference)
"""Trainium2 Bass kernel for nn_AttentionHead (B=2, S=2048, D=1024, H=16).

Sharding: 8 cores = 2 batches x 4 head-groups (4 heads each).
Per core (batch b, heads h0..h0+3):
  - QT/KT = (Wq|k_aug).T @ x_aug.T   (biases via ones-row; 1/sqrt(hd) folded in Wq)
  - V natural [m, hd+1] with an appended ones column (gives softmax col-sums free)
  - scoresT tiles [m=128, n=512] -> exp on ScalarE -> AV accumulates
    V_aug.T @ expST -> [hd+1, n]; row hd = softmax denominators
  - denominators: DVE reciprocal -> DRAM bounce -> gpsimd partition-broadcast
    load -> DVE normalize (keeps the PE stream free of softmax dependencies)
  - weightsT per head written [m, n]; host transposes into [B,S,S,H]
  - out partial = outT.T @ Wo_rows; host sums the 4 partials per batch (+bo)
All matmul operands are float32r (TF32-like, full PE rate at N>=256).
"""

import sys

try:
    import concourse  # noqa: F401  (already on path via axon site boot)
except ImportError:
    sys.path.append("/opt/trn_rl_repo")

import numpy as np

S, D, H, HD = 2048, 1024, 16, 64
HPC = 4            # heads per core
P = 128
NC = 512           # n-chunk (free-dim tile for scoresT / weights)
KT = D // P        # 8 contraction tiles for projections
NT = S // P        # 16 m-tiles (keys)
DH = HPC * HD      # 256 head-group width
DV = HD + 1        # 65 = value dims + ones column

_cache = {}


def _build(s=S, trace=False):
    import concourse.bass as bass
    import concourse.tile as tile
    from concourse import bacc, mybir
    from concourse.masks import make_identity

    f32 = mybir.dt.float32
    f32r = mybir.dt.float32r
    AF = mybir.ActivationFunctionType
    nt = s // P
    nch = s // NC

    nc = bacc.Bacc("TRN2", target_bir_lowering=False, debug=False, num_devices=8)

    x = nc.dram_tensor("x", [s, D], f32, kind="ExternalInput").ap()
    wq = nc.dram_tensor("wq", [D + 1, DH], f32r, kind="ExternalInput").ap()
    wk = nc.dram_tensor("wk", [D + 1, DH], f32r, kind="ExternalInput").ap()
    wv = nc.dram_tensor("wv", [D + 1, HPC * DV], f32r, kind="ExternalInput").ap()
    wo = nc.dram_tensor("wo", [DH, D], f32r, kind="ExternalInput").ap()
    wts = nc.dram_tensor("wts", [HPC, s, s], f32, kind="ExternalOutput").ap()
    outp = nc.dram_tensor("outp", [s, D], f32, kind="ExternalOutput").ap()
    # per-(head,chunk) softmax denominator + reciprocal bounce buffers
    dsc = nc.dram_tensor("dsc", [HPC * nch, NC], f32, kind="Internal").ap()
    dsc2 = nc.dram_tensor("dsc2", [HPC * nch, NC], f32, kind="Internal").ap()

    with tile.TileContext(nc) as tc:
        with (
            tc.tile_pool(name="const", bufs=1) as constp,
            tc.tile_pool(name="wpool", bufs=1) as wpool,
            tc.tile_pool(name="xa", bufs=1) as xap,
            tc.tile_pool(name="xt", bufs=1) as xtp,
            tc.tile_pool(name="qkv", bufs=1) as qkvp,
            tc.tile_pool(name="chunk", bufs=2) as chp,
            tc.tile_pool(name="small", bufs=2) as smp,
            tc.tile_pool(name="obuf", bufs=1) as obp,
            tc.tile_pool(name="nm", bufs=2) as nmp,
            tc.tile_pool(name="ps", bufs=3, space="PSUM") as pp,
            tc.tile_pool(name="psav", bufs=2, space="PSUM") as pav,
        ):
            ident = constp.tile([P, P], f32)
            make_identity(nc, ident)
            ones32 = constp.tile([1, NC], f32)
            nc.vector.memset(ones32, 1.0)
            ones1 = constp.tile([1, NC], f32r)
            nc.vector.tensor_copy(ones1, ones32)

            wq_sb = wpool.tile([P, KT, DH], f32r, tag="wq")
            wk_sb = wpool.tile([P, KT, DH], f32r, tag="wk")
            wv_sb = wpool.tile([P, KT, HPC * DV], f32r, tag="wv")
            wqb = wpool.tile([1, DH], f32r, tag="wqb")
            wkb = wpool.tile([1, DH], f32r, tag="wkb")
            wvb = wpool.tile([1, HPC * DV], f32r, tag="wvb")
            for k in range(KT):
                nc.sync.dma_start(wq_sb[:, k, :], wq[k * P : (k + 1) * P, :])
                nc.sync.dma_start(wk_sb[:, k, :], wk[k * P : (k + 1) * P, :])
                nc.sync.dma_start(wv_sb[:, k, :], wv[k * P : (k + 1) * P, :])
            nc.sync.dma_start(wqb, wq[D : D + 1, :])
            nc.sync.dma_start(wkb, wk[D : D + 1, :])
            nc.sync.dma_start(wvb, wv[D : D + 1, :])

            # persistent projected tensors: [d-on-partitions, n] layout
            qt_sb = [qkvp.tile([P, s], f32r, tag=f"qt{i}", name=f"qt{i}") for i in range(2)]
            kt_sb = [qkvp.tile([P, s], f32r, tag=f"kt{i}", name=f"kt{i}") for i in range(2)]
            ot_sb = [qkvp.tile([P, s], f32r, tag=f"ot{i}", name=f"ot{i}") for i in range(2)]
            v_sb = qkvp.tile([P, nt, HPC * DV], f32r, tag="v")

            # ---- phase 1: projections (per n-chunk of NC positions) ----
            for ch in range(nch):
                npt = NC // P  # n-tiles per chunk
                xt = xtp.tile([P, KT, NC], f32r, tag="xt")
                for t in range(npt):
                    n0 = ch * NC + t * P
                    xa = xap.tile([P, D], f32, tag="xa", bufs=2)
                    nc.sync.dma_start(xa, x[n0 : n0 + P, :])
                    for k in range(KT):
                        ps = pp.tile([P, 2, NC], f32, tag="ps")
                        nc.tensor.transpose(ps[:, 0, 0:P], xa[:, k * P : (k + 1) * P], ident)
                        nc.vector.tensor_copy(xt[:, k, t * P : (t + 1) * P], ps[:, 0, 0:P])
                cs = slice(ch * NC, (ch + 1) * NC)
                for j in range(2):
                    psq = pp.tile([P, 2, NC], f32, tag="ps")
                    psk = pp.tile([P, 2, NC], f32, tag="ps")
                    js = slice(j * P, (j + 1) * P)
                    for k in range(KT):
                        nc.tensor.matmul(psq[:, 0, :], wq_sb[:, k, js], xt[:, k, :],
                                         start=(k == 0), stop=False)
                    nc.tensor.matmul(psq[:, 0, :], wqb[:, js], ones1,
                                     start=False, stop=True)
                    nc.vector.tensor_copy(qt_sb[j][:, cs], psq[:, 0, :])
                    for k in range(KT):
                        nc.tensor.matmul(psk[:, 0, :], wk_sb[:, k, js], xt[:, k, :],
                                         start=(k == 0), stop=False)
                    nc.tensor.matmul(psk[:, 0, :], wkb[:, js], ones1,
                                     start=False, stop=True)
                    nc.vector.tensor_copy(kt_sb[j][:, cs], psk[:, 0, :])
                for t in range(npt):
                    mt = ch * npt + t
                    psv = pp.tile([P, 2, NC], f32, tag="ps")
                    ts = slice(t * P, (t + 1) * P)
                    for k in range(KT):
                        nc.tensor.matmul(psv[:, 0, 0 : HPC * DV], xt[:, k, ts],
                                         wv_sb[:, k, :], start=(k == 0), stop=False)
                    nc.tensor.matmul(psv[:, 0, 0 : HPC * DV], ones1[:, 0:P], wvb,
                                     start=False, stop=True)
                    nc.vector.tensor_copy(v_sb[:, mt, :], psv[:, 0, 0 : HPC * DV])

            # Wo loads reuse the xt slot (dead after phase 1)
            wo_sb = xtp.tile([P, DH // P, D], f32r, tag="xt", name="wo_sb")
            for j in range(DH // P):
                nc.sync.dma_start(wo_sb[:, j, :], wo[j * P : (j + 1) * P, :])

            # warm-up burst: full-array bf16 MMs re-trip the HAM activity
            wb16 = constp.tile([P, NC], mybir.dt.bfloat16)
            nc.vector.memset(wb16, 1.0)

            def warm(n=6):
                for _ in range(n):
                    psw = pp.tile([P, 2, NC], f32, tag="ps")
                    nc.tensor.matmul(psw[:, 0, :], wb16[:, 0:P], wb16,
                                     start=True, stop=True)

            warm()

            # ---- phase 2: attention, chunk-major, head-PAIRED (full-array
            # row-group concurrency), tails software-pipelined ----
            NW = max(1, min(4, nt // 4))  # w chunk split for early slot release
            HT = nt // NW  # m-tiles per quarter-chunk buffer

            def tail(h, ch, wh, av):
                g, o = h // 2, (h % 2) * HD
                cs = slice(ch * NC, (ch + 1) * NC)
                idx = h * nch + ch
                # sums row -> DRAM -> [128,4] view -> reciprocal on 128 lanes
                sms = smp.tile([1, NC], f32, tag="sms", bufs=2)
                nc.scalar.copy(sms, av[HD : HD + 1, :])
                nc.sync.dma_start(dsc[idx : idx + 1, :], sms)
                drow = dsc[idx : idx + 1, :]
                rsq = bass.AP(tensor=drow.tensor, offset=drow.offset,
                              ap=[[NC // P, P], [1, NC // P]])
                rsm = smp.tile([P, NC // P], f32, tag="rsm", bufs=2)
                nc.gpsimd.dma_start(rsm, rsq)
                nc.vector.reciprocal(rsm, rsm)
                drow2 = dsc2[idx : idx + 1, :]
                rsq2 = bass.AP(tensor=drow2.tensor, offset=drow2.offset,
                               ap=[[NC // P, P], [1, NC // P]])
                nc.sync.dma_start(rsq2, rsm)
                bcs = smp.tile([P, NC], f32, tag="bcs")
                bsrc = bass.AP(tensor=drow2.tensor, offset=drow2.offset,
                               ap=[[0, P], [1, NC]])
                nc.gpsimd.dma_start(bcs, bsrc)
                # normalized attention output chunk (pre-Wo)
                nc.vector.tensor_mul(ot_sb[g][o : o + HD, cs], av[0:HD, :],
                                     bcs[0:HD, :])
                # normalized weights -> nm buffers -> DRAM
                for q in range(NW):
                    w = wh[q]
                    for mb in range(HT // 2):
                        bb = bass.AP(tensor=bcs.tensor, offset=bcs.offset,
                                     ap=[bcs.ap[0], [0, 2], [1, NC]])
                        nm = nmp.tile([P, 2, NC], f32, tag="nm")
                        nc.vector.tensor_mul(
                            nm, w[:, 2 * mb : 2 * mb + 2, :].bitcast(f32), bb)
                        m0 = (q * HT + 2 * mb) * P
                        dst = wts[h, m0 : m0 + 2 * P, cs]
                        nc.sync.dma_start(dst.rearrange("(j p) c -> p j c", p=P),
                                          nm)

            def wo_proj(ch):
                for t in range(ch * (NC // P), (ch + 1) * (NC // P)):
                    os_ = obp.tile([P, D], f32, tag="os")
                    ts = slice(t * P, (t + 1) * P)
                    for c in range(2):
                        po = pp.tile([P, 2, NC], f32, tag="ps")
                        ncs = slice(c * NC, (c + 1) * NC)
                        for j in range(2):
                            nc.tensor.matmul(po[:, 0, :], ot_sb[j][:, ts],
                                             wo_sb[:, j, ncs],
                                             start=(j == 0), stop=(j == 1))
                        nc.vector.tensor_copy(os_[:, ncs], po[:, 0, :])
                    nc.sync.dma_start(outp[ts, :], os_)

            pending = []
            for ch in range(nch):
                cs = slice(ch * NC, (ch + 1) * NC)
                for g in range(HPC // 2):
                    ha, hb = 2 * g, 2 * g + 1
                    wa = [chp.tile([P, HT, NC], f32r, tag=f"wa{q}",
                                   name=f"wa{q}", bufs=1) for q in range(NW)]
                    wb_ = [chp.tile([P, HT, NC], f32r, tag=f"wb{q}",
                                    name=f"wb{q}", bufs=1) for q in range(NW)]
                    ava = pav.tile([DV, NC], f32, tag="ava", bufs=1)
                    avb = pav.tile([DV, NC], f32, tag="avb", bufs=1)
                    for mt in range(nt):
                        q, wt = mt // HT, mt % HT
                        ps = pp.tile([P, 2, NC], f32, tag="ps")
                        nc.tensor.matmul(
                            ps[:, 0, :],
                            kt_sb[g][0:HD, mt * P : (mt + 1) * P],
                            qt_sb[g][0:HD, cs],
                            start=True, stop=True,
                        )
                        nc.tensor.matmul(
                            ps[:, 1, :],
                            kt_sb[g][HD : 2 * HD, mt * P : (mt + 1) * P],
                            qt_sb[g][HD : 2 * HD, cs],
                            start=True, stop=True,
                        )
                        nc.scalar.activation(wa[q][:, wt, :], ps[:, 0, :], AF.Exp)
                        nc.scalar.activation(wb_[q][:, wt, :], ps[:, 1, :], AF.Exp)
                        nc.tensor.matmul(
                            ava, v_sb[:, mt, ha * DV : (ha + 1) * DV],
                            wa[q][:, wt, :],
                            start=(mt == 0), stop=(mt == nt - 1),
                            skip_group_check=True,
                        )
                        nc.tensor.matmul(
                            avb, v_sb[:, mt, hb * DV : (hb + 1) * DV],
                            wb_[q][:, wt, :],
                            start=(mt == 0), stop=(mt == nt - 1),
                            skip_group_check=True,
                        )
                    if pending:
                        for t in pending.pop(0):
                            tail(*t)
                        if g == 0 and ch > 0:
                            wo_proj(ch - 1)
                    pending.append([(ha, ch, wa, ava), (hb, ch, wb_, avb)])
            for grp in pending:
                for t in grp:
                    tail(*t)
            wo_proj(nch - 1)

    nc.compile()
    return nc


def _prep_in_maps(queries, Wq, bq, Wkv, bkv, Wo, bo):
    queries = np.asarray(queries, np.float32)
    Wq = np.asarray(Wq, np.float32)
    bq = np.asarray(bq, np.float32)
    Wkv = np.asarray(Wkv, np.float32)
    bkv = np.asarray(bkv, np.float32)
    Wo = np.asarray(Wo, np.float32)
    in_maps = []
    for c in range(8):
        b, h0 = c // 4, (c % 4) * HPC
        cols = slice(h0 * HD, (h0 + HPC) * HD)
        wq_aug = np.concatenate([Wq[:, cols], bq[cols][None, :]], axis=0) * (HD ** -0.5)
        wk_aug = np.concatenate([Wkv[:, cols], bkv[cols][None, :]], axis=0)
        wv_aug = np.zeros((D + 1, HPC * DV), np.float32)
        for j in range(HPC):
            h = h0 + j
            wv_aug[:D, j * DV : j * DV + HD] = Wkv[:, D + h * HD : D + (h + 1) * HD]
            wv_aug[D, j * DV : j * DV + HD] = bkv[D + h * HD : D + (h + 1) * HD]
            wv_aug[D, j * DV + HD] = 1.0
        in_maps.append({
            "x": np.ascontiguousarray(queries[b]),
            "wq": np.ascontiguousarray(wq_aug, np.float32),
            "wk": np.ascontiguousarray(wk_aug, np.float32),
            "wv": wv_aug,
            "wo": np.ascontiguousarray(Wo[cols, :], np.float32),
        })
    return in_maps


def _gather(results, bo):
    bo = np.asarray(bo, np.float32)
    weights = np.empty((2, S, S, H), np.float32)
    out = np.zeros((2, S, D), np.float32)
    for c in range(8):
        b, h0 = c // 4, (c % 4) * HPC
        wc = results[c]["wts"]                      # [HPC, m, n]
        weights[b, :, :, h0 : h0 + HPC] = wc.transpose(2, 1, 0)
        out[b] += results[c]["outp"]
    out += bo[None, None, :]
    return out, weights


def run(inputs, trace=False, tmpdir=None):
    from concourse.bass_utils import run_bass_kernel_spmd

    if "nc" not in _cache:
        _cache["nc"] = _build()
    nc = _cache["nc"]
    in_maps = _prep_in_maps(
        inputs["queries"], inputs["Wq"], inputs["bq"], inputs["Wkv"],
        inputs["bkv"], inputs["Wo"], inputs["bo"],
    )
    res = run_bass_kernel_spmd(
        nc, in_maps, core_ids=list(range(8)), trace=trace, tmpdir=tmpdir
    )
    return _gather(res.results, inputs["bo"]), res


def kernel(**inputs):
    return run(inputs)[0]


# revision 31
# speedup vs baseline: 1.3059x; 1.3059x over previous
"""Trainium2 Bass kernel for nn_AttentionHead (B=2, S=2048, D=1024, H=16).

Sharding: 8 cores = 2 batches x 4 head-groups (4 heads each).
Per core (batch b, heads h0..h0+3):
  - QT/KT = (Wq|k_aug).T @ x_aug.T   (biases via ones-row; 1/sqrt(hd) folded in Wq)
  - V natural [m, hd+1] with an appended ones column (gives softmax col-sums free)
  - scoresT tiles [m=128, n=512] -> exp on ScalarE -> AV accumulates
    V_aug.T @ expST -> [hd+1, n]; row hd = softmax denominators
  - denominators: DVE reciprocal -> DRAM bounce -> gpsimd partition-broadcast
    load -> DVE normalize (keeps the PE stream free of softmax dependencies)
  - weightsT per head written [m, n]; host transposes into [B,S,S,H]
  - out partial = outT.T @ Wo_rows; host sums the 4 partials per batch (+bo)
All matmul operands are float32r (TF32-like, full PE rate at N>=256).
"""

import sys

try:
    import concourse  # noqa: F401  (already on path via axon site boot)
except ImportError:
    sys.path.append("/opt/trn_rl_repo")

import numpy as np

S, D, H, HD = 2048, 1024, 16, 64
HPC = 4            # heads per core
P = 128
NC = 512           # n-chunk (free-dim tile for scoresT / weights)
KT = D // P        # 8 contraction tiles for projections
NT = S // P        # 16 m-tiles (keys)
DH = HPC * HD      # 256 head-group width
DV = HD + 1        # 65 = value dims + ones column

_cache = {}


def _build(s=S, trace=False):
    import concourse.bass as bass
    import concourse.tile as tile
    from concourse import bacc, mybir

    f32 = mybir.dt.float32
    f32r = mybir.dt.float32r
    AF = mybir.ActivationFunctionType
    nt = s // P
    nch = s // NC

    nc = bacc.Bacc("TRN2", target_bir_lowering=False, debug=False, num_devices=8)

    xtr = nc.dram_tensor("xtr", [D, s], f32r, kind="ExternalInput").ap()
    wq = nc.dram_tensor("wq", [D + 1, DH], f32r, kind="ExternalInput").ap()
    wk = nc.dram_tensor("wk", [D + 1, DH], f32r, kind="ExternalInput").ap()
    wv = nc.dram_tensor("wv", [D + 1, HPC * DV], f32r, kind="ExternalInput").ap()
    wo = nc.dram_tensor("wo", [DH, D], f32r, kind="ExternalInput").ap()
    wts = nc.dram_tensor("wts", [HPC, s, s], f32, kind="ExternalOutput").ap()
    outp = nc.dram_tensor("outp", [s, D], f32, kind="ExternalOutput").ap()
    # per-(head,chunk) softmax denominator + reciprocal bounce buffers
    dsc = nc.dram_tensor("dsc", [HPC * nch, NC], f32, kind="Internal").ap()
    dsc2 = nc.dram_tensor("dsc2", [HPC * nch, NC], f32, kind="Internal").ap()

    with tile.TileContext(nc) as tc:
        with (
            tc.tile_pool(name="const", bufs=1) as constp,
            tc.tile_pool(name="wpool", bufs=1) as wpool,
            tc.tile_pool(name="xt", bufs=1) as xtp,
            tc.tile_pool(name="qkv", bufs=1) as qkvp,
            tc.tile_pool(name="chunk", bufs=2) as chp,
            tc.tile_pool(name="small", bufs=2) as smp,
            tc.tile_pool(name="obuf", bufs=2) as obp,
            tc.tile_pool(name="ps", bufs=3, space="PSUM") as pp,
            tc.tile_pool(name="psav", bufs=2, space="PSUM") as pav,
        ):
            ones32 = constp.tile([1, NC], f32)
            nc.vector.memset(ones32, 1.0)
            ones1 = constp.tile([1, NC], f32r)
            nc.vector.tensor_copy(ones1, ones32)

            wq_sb = wpool.tile([P, KT, DH], f32r, tag="wq")
            wk_sb = wpool.tile([P, KT, DH], f32r, tag="wk")
            wv_sb = wpool.tile([P, KT, HPC * DV], f32r, tag="wv")
            wqb = wpool.tile([1, DH], f32r, tag="wqb")
            wkb = wpool.tile([1, DH], f32r, tag="wkb")
            wvb = wpool.tile([1, HPC * DV], f32r, tag="wvb")
            # issue order matters: the first V matmuls need only wv + the
            # first xt chunk, so load those ahead of wq/wk in the queue
            for k in range(KT):
                nc.sync.dma_start(wv_sb[:, k, :], wv[k * P : (k + 1) * P, :])
            nc.sync.dma_start(wvb, wv[D : D + 1, :])
            for k in range(KT):
                nc.sync.dma_start(wq_sb[:, k, :], wq[k * P : (k + 1) * P, :])
                nc.sync.dma_start(wk_sb[:, k, :], wk[k * P : (k + 1) * P, :])
            nc.sync.dma_start(wqb, wq[D : D + 1, :])
            nc.sync.dma_start(wkb, wk[D : D + 1, :])

            # persistent projected tensors: [d-on-partitions, n] layout
            qt_sb = [qkvp.tile([P, s], f32r, tag=f"qt{i}", name=f"qt{i}") for i in range(2)]
            kt_sb = [qkvp.tile([P, s], f32r, tag=f"kt{i}", name=f"kt{i}") for i in range(2)]
            ot_sb = [qkvp.tile([P, s], f32r, tag=f"ot{i}", name=f"ot{i}") for i in range(2)]
            v_sb = qkvp.tile([P, nt, HPC * DV], f32r, tag="v")

            # ---- phase 1: projections (per n-chunk of NC positions);
            # x arrives pre-transposed from the host, loaded straight as f32r
            for ch in range(nch):
                npt = NC // P  # n-tiles per chunk
                cs = slice(ch * NC, (ch + 1) * NC)
                xt = xtp.tile([P, KT, NC], f32r, tag="xt")
                for k in range(KT):
                    dma_eng = nc.gpsimd if ch == 0 else nc.sync
                    dma_eng.dma_start(xt[:, k, :], xtr[k * P : (k + 1) * P, cs])
                for t in range(npt):
                    mt = ch * npt + t
                    psv = pp.tile([P, 2, NC], f32, tag="ps")
                    ts = slice(t * P, (t + 1) * P)
                    for k in range(KT):
                        nc.tensor.matmul(psv[:, 0, 0 : HPC * DV], xt[:, k, ts],
                                         wv_sb[:, k, :], start=(k == 0), stop=False)
                    nc.tensor.matmul(psv[:, 0, 0 : HPC * DV], ones1[:, 0:P], wvb,
                                     start=False, stop=True)
                    nc.vector.tensor_copy(v_sb[:, mt, :], psv[:, 0, 0 : HPC * DV])
                for j in range(2):
                    psq = pp.tile([P, 2, NC], f32, tag="ps")
                    psk = pp.tile([P, 2, NC], f32, tag="ps")
                    js = slice(j * P, (j + 1) * P)
                    for k in range(KT):
                        nc.tensor.matmul(psq[:, 0, :], wq_sb[:, k, js], xt[:, k, :],
                                         start=(k == 0), stop=False)
                    nc.tensor.matmul(psq[:, 0, :], wqb[:, js], ones1,
                                     start=False, stop=True)
                    nc.vector.tensor_copy(qt_sb[j][:, cs], psq[:, 0, :])
                    for k in range(KT):
                        nc.tensor.matmul(psk[:, 0, :], wk_sb[:, k, js], xt[:, k, :],
                                         start=(k == 0), stop=False)
                    nc.tensor.matmul(psk[:, 0, :], wkb[:, js], ones1,
                                     start=False, stop=True)
                    nc.vector.tensor_copy(kt_sb[j][:, cs], psk[:, 0, :])

            # Wo loads reuse the xt slot (dead after phase 1)
            wo_sb = xtp.tile([P, DH // P, D], f32r, tag="xt", name="wo_sb")
            for j in range(DH // P):
                nc.sync.dma_start(wo_sb[:, j, :], wo[j * P : (j + 1) * P, :])

            # ---- phase 2: attention, chunk-major, tails software-pipelined ----
            NW = 1  # single chunk buffer (quarter-split measured slower)
            HT = nt // NW  # m-tiles per quarter-chunk buffer

            def tail(h, ch, wh, av):
                g, o = h // 2, (h % 2) * HD
                cs = slice(ch * NC, (ch + 1) * NC)
                idx = h * nch + ch
                recip = smp.tile([1, NC], f32, tag="recip", bufs=2)
                nc.vector.reciprocal(recip, av[HD : HD + 1, :])
                drow2 = dsc2[idx : idx + 1, :]
                nc.sync.dma_start(drow2, recip)
                bcs = smp.tile([P, NC], f32, tag="bcs")
                bsrc = bass.AP(tensor=drow2.tensor, offset=drow2.offset,
                               ap=[[0, P], [1, NC]])
                nc.gpsimd.dma_start(bcs, bsrc)
                # normalized attention output chunk (pre-Wo)
                nc.vector.tensor_mul(ot_sb[g][o : o + HD, cs], av[0:HD, :],
                                     bcs[0:HD, :])
                # normalized weights in-place (f32r-quantized) -> DRAM
                for q in range(NW):
                    w = wh[q]
                    for mb in range(HT // 2):
                        bb = bass.AP(tensor=bcs.tensor, offset=bcs.offset,
                                     ap=[bcs.ap[0], [0, 2], [1, NC]])
                        wsl = w[:, 2 * mb : 2 * mb + 2, :]
                        nc.vector.tensor_mul(wsl, wsl.bitcast(f32), bb)
                        m0 = (q * HT + 2 * mb) * P
                        dst = wts[h, m0 : m0 + 2 * P, cs]
                        nc.sync.dma_start(dst.rearrange("(j p) c -> p j c", p=P),
                                          wsl.bitcast(f32))

            def wo_proj(ch):
                for t in range(ch * (NC // P), (ch + 1) * (NC // P)):
                    os_ = obp.tile([P, D], f32, tag="os")
                    ts = slice(t * P, (t + 1) * P)
                    for c in range(2):
                        po = pp.tile([P, 2, NC], f32, tag="ps")
                        ncs = slice(c * NC, (c + 1) * NC)
                        for j in range(2):
                            nc.tensor.matmul(po[:, 0, :], ot_sb[j][:, ts],
                                             wo_sb[:, j, ncs],
                                             start=(j == 0), stop=(j == 1))
                        nc.vector.tensor_copy(os_[:, ncs], po[:, 0, :])
                    nc.sync.dma_start(outp[ts, :], os_)

            pending = None
            for ch in range(nch):
                cs = slice(ch * NC, (ch + 1) * NC)
                for h in range(HPC):
                    g, o = h // 2, (h % 2) * HD
                    wh = [chp.tile([P, HT, NC], f32r, tag=f"w{q}",
                                   name=f"w{q}", bufs=2) for q in range(NW)]
                    av = pav.tile([DV, NC], f32, tag="av")
                    for mt in range(nt):
                        q, wt = mt // HT, mt % HT
                        ps = pp.tile([P, 2, NC], f32, tag="ps")
                        nc.tensor.matmul(
                            ps[:, 0, :],
                            kt_sb[g][o : o + HD, mt * P : (mt + 1) * P],
                            qt_sb[g][o : o + HD, cs],
                            start=True, stop=True,
                        )
                        nc.scalar.activation(wh[q][:, wt, :], ps[:, 0, :], AF.Exp)
                        nc.tensor.matmul(
                            av, v_sb[:, mt, h * DV : (h + 1) * DV],
                            wh[q][:, wt, :],
                            start=(mt == 0), stop=(mt == nt - 1),
                            skip_group_check=True,
                        )
                    if pending is not None:
                        tail(*pending)
                        ph, pch = pending[0], pending[1]
                        if ph == HPC - 1:
                            wo_proj(pch)
                    pending = (h, ch, wh, av)
            tail(*pending)
            wo_proj(nch - 1)

    nc.compile()
    return nc


def _prep_in_maps(queries, Wq, bq, Wkv, bkv, Wo, bo):
    queries = np.asarray(queries, np.float32)
    Wq = np.asarray(Wq, np.float32)
    bq = np.asarray(bq, np.float32)
    Wkv = np.asarray(Wkv, np.float32)
    bkv = np.asarray(bkv, np.float32)
    Wo = np.asarray(Wo, np.float32)
    in_maps = []
    for c in range(8):
        b, h0 = c // 4, (c % 4) * HPC
        cols = slice(h0 * HD, (h0 + HPC) * HD)
        wq_aug = np.concatenate([Wq[:, cols], bq[cols][None, :]], axis=0) * (HD ** -0.5)
        wk_aug = np.concatenate([Wkv[:, cols], bkv[cols][None, :]], axis=0)
        wv_aug = np.zeros((D + 1, HPC * DV), np.float32)
        for j in range(HPC):
            h = h0 + j
            wv_aug[:D, j * DV : j * DV + HD] = Wkv[:, D + h * HD : D + (h + 1) * HD]
            wv_aug[D, j * DV : j * DV + HD] = bkv[D + h * HD : D + (h + 1) * HD]
            wv_aug[D, j * DV + HD] = 1.0
        in_maps.append({
            "xtr": np.ascontiguousarray(queries[b].T),
            "wq": np.ascontiguousarray(wq_aug, np.float32),
            "wk": np.ascontiguousarray(wk_aug, np.float32),
            "wv": wv_aug,
            "wo": np.ascontiguousarray(Wo[cols, :], np.float32),
        })
    return in_maps


def _gather(results, bo):
    bo = np.asarray(bo, np.float32)
    weights = np.empty((2, S, S, H), np.float32)
    out = np.zeros((2, S, D), np.float32)
    for c in range(8):
        b, h0 = c // 4, (c % 4) * HPC
        wc = results[c]["wts"]                      # [HPC, m, n]
        weights[b, :, :, h0 : h0 + HPC] = wc.transpose(2, 1, 0)
        out[b] += results[c]["outp"]
    out += bo[None, None, :]
    return out, weights


def run(inputs, trace=False, tmpdir=None):
    from concourse.bass_utils import run_bass_kernel_spmd

    if "nc" not in _cache:
        _cache["nc"] = _build()
    nc = _cache["nc"]
    in_maps = _prep_in_maps(
        inputs["queries"], inputs["Wq"], inputs["bq"], inputs["Wkv"],
        inputs["bkv"], inputs["Wo"], inputs["bo"],
    )
    res = run_bass_kernel_spmd(
        nc, in_maps, core_ids=list(range(8)), trace=trace, tmpdir=tmpdir
    )
    return _gather(res.results, inputs["bo"]), res


def kernel(**inputs):
    return run(inputs)[0]
